# revision 21
# baseline (speedup 1.0000x reference)
"""Trainium2 Bass kernel for nn_CrossStockRelationship.

Computation (reference):
    rel_encoded = MLP(relationship_matrix[stock_idx])      # [S, H], tiny
    rel_encoded[stock_idx] = 0                             # mask
    out[b, h]  = sum_s encoded_states[b, s, h] * rel_encoded[s, h]

The einsum over the 512 MB encoded_states tensor is the entire cost
(memory-bound). Strategy: shard the S (stock) axis over the 8 cores
(250 stocks each); every core reads all 1024 batches for its stock
slice (64 KB contiguous per batch row -> full-rate DMA) and produces a
partial [1024, 64] output; the host sums the 8 partials. The tiny MLP
(0.006% of FLOPs) runs on host; its [250, 64] result is broadcast
across the 128 SBUF partitions on device and multiplied elementwise
against batch-major tiles, then reduced over s on the vector engine.
"""

import os
import sys

for _p in ("/opt/trn_rl_repo", "/root/.axon_site/_ro/trn_rl_repo"):
    if os.path.isdir(_p) and _p not in sys.path:
        sys.path.insert(0, _p)

import numpy as np

import concourse.bass as bass
import concourse.bacc as bacc
import concourse.tile as tile
from concourse import mybir
from concourse.bass_utils import run_bass_kernel_spmd

N_CORES = 8
B = 1024
S = 2000
H = 64
S_PER = S // N_CORES  # 250
P = 128
N_BTILES = B // P  # 8
F = S_PER * H  # 16000 floats = 64 KB per partition

# PATH "A": tensor_mul into a (h, s)-transposed prod buffer + tensor_reduce
#           over the contiguous innermost s axis, per s-chunk.
# PATH "B": 64 fused tensor_tensor_reduce ops per batch tile (one per h),
#           no prod buffer.
PATH = os.environ.get("KERNEL_PATH", "B")
S_CHUNKS = (126, 124)  # even sizes (2x perf mode requires even innermost dim)

TRACE = False  # set by test.py; run_bass_kernel_spmd also honors BASS_TRACE
LAST_RESULT = None

_NC_CACHE = {}


def _replicate_row(nc, rel_bcast, rel_h, col_splits):
    # Stage the [1, F] row into partition 0 (64 KB HBM read), then
    # replicate to all 128 partitions by log-doubling SBUF->SBUF copies.
    # Bacc's generate_event_semaphores splits the multi-sem waits this
    # join creates into legal single-wait instructions.
    if os.environ.get("KERNEL_DRAM_BCAST", "1") == "1":
        c0 = 0
        for cw in col_splits:
            nc.gpsimd.dma_start(
                out=rel_bcast[:, c0 : c0 + cw],
                in_=rel_h[0:1, c0 : c0 + cw].broadcast_to([P, cw]),
            )
            c0 += cw
        return
    nc.sync.dma_start(out=rel_bcast[0:1, :], in_=rel_h[:, :])
    p = 1
    while p < P:
        n = min(p, P - p)
        nc.sync.dma_start(out=rel_bcast[p : p + n, :], in_=rel_bcast[0:n, :])
        p += n


def _build_path_a(nc, tc, enc_h, rel_h, out_h, ctx):
    f32 = mybir.dt.float32
    bcast_pool = ctx.enter_context(tc.tile_pool(name="bcast", bufs=1))
    enc_pool = ctx.enter_context(tc.tile_pool(name="enc", bufs=2))
    prod_pool = ctx.enter_context(tc.tile_pool(name="prod", bufs=2))
    small_pool = ctx.enter_context(tc.tile_pool(name="small", bufs=6))

    rel_bcast = bcast_pool.tile([P, F], f32)
    _replicate_row(nc, rel_bcast, rel_h, [cs * H for cs in S_CHUNKS])
    rel_3d = rel_bcast[:, :].rearrange("p (s h) -> p s h", h=H)

    for ib in range(N_BTILES):
        accs = []
        s0 = 0
        for cs in S_CHUNKS:
            et = enc_pool.tile([P, cs, H], f32, tag="enc")
            nc.sync.dma_start(
                out=et[:, :, :],
                in_=enc_h[ib * P : (ib + 1) * P, s0 : s0 + cs, :],
            )
            pt = prod_pool.tile([P, H, cs], f32, tag="prod")
            nc.vector.tensor_mul(
                pt[:, :, :].rearrange("p h s -> p s h"),
                et[:, :, :],
                rel_3d[:, s0 : s0 + cs, :],
            )
            acc = small_pool.tile([P, H], f32, tag="acc")
            nc.vector.reduce_sum(
                out=acc[:, :], in_=pt[:, :, :], axis=mybir.AxisListType.X
            )
            accs.append(acc)
            s0 += cs
        ot = small_pool.tile([P, H], f32, tag="ot")
        nc.vector.tensor_add(ot[:, :], accs[0][:, :], accs[1][:, :])
        nc.sync.dma_start(out=out_h[ib * P : (ib + 1) * P, :], in_=ot[:, :])


def _build_path_b(nc, tc, enc_h, rel_h, out_h, ctx):
    f32 = mybir.dt.float32
    bcast_pool = ctx.enter_context(tc.tile_pool(name="bcast", bufs=1))
    enc_pool = ctx.enter_context(tc.tile_pool(name="enc", bufs=2))
    small_pool = ctx.enter_context(tc.tile_pool(name="small", bufs=4))

    rel_bcast = bcast_pool.tile([P, F], f32)
    _replicate_row(nc, rel_bcast, rel_h, [F])
    rel_3d = rel_bcast[:, :].rearrange("p (s h) -> p h s", h=H)

    for ib in range(N_BTILES):
        et = enc_pool.tile([P, F], f32, tag="enc")
        nc.sync.dma_start(
            out=et[:, :], in_=enc_h[ib * P : (ib + 1) * P, :, :]
        )
        et_3d = et[:, :].rearrange("p (s h) -> p h s", h=H)
        ot = small_pool.tile([P, H], f32, tag="ot")
        # gpsimd offload of some per-h ops sims faster but the
        # TensorScalarPtr opcode is rejected on POOL by walrus codegen;
        # keep it off by default.
        k_pool = int(os.environ.get("KERNEL_KPOOL", "0"))
        for h in range(H):
            if h >= H - k_pool:
                scratch = small_pool.tile([P, S_PER], f32, tag=f"scrp{h % 2}")
                nc.gpsimd.scalar_tensor_tensor(
                    out=scratch[:, :],
                    in0=et_3d[:, h, :],
                    scalar=0.0,
                    in1=rel_3d[:, h, :],
                    op0=mybir.AluOpType.bypass,
                    op1=mybir.AluOpType.mult,
                    accum_out=ot[:, h : h + 1],
                )
            elif os.environ.get("KERNEL_TTR", "0") == "1":
                scratch = small_pool.tile([P, S_PER], f32, tag=f"scrv{h % 2}")
                nc.vector.tensor_tensor_reduce(
                    out=scratch[:, :],
                    in0=et_3d[:, h, :],
                    in1=rel_3d[:, h, :],
                    scale=1.0,
                    scalar=0.0,
                    op0=mybir.AluOpType.mult,
                    op1=mybir.AluOpType.add,
                    accum_out=ot[:, h : h + 1],
                )
            else:
                scratch = small_pool.tile([P, S_PER], f32, tag=f"scrv{h % 2}")
                nc.vector.scalar_tensor_tensor(
                    out=scratch[:, :],
                    in0=et_3d[:, h, :],
                    scalar=0.0,
                    in1=rel_3d[:, h, :],
                    op0=mybir.AluOpType.bypass,
                    op1=mybir.AluOpType.mult,
                    accum_out=ot[:, h : h + 1],
                )
        nc.sync.dma_start(out=out_h[ib * P : (ib + 1) * P, :], in_=ot[:, :])


def _build_path_c(nc, tc, enc_h, rel_h, out_h, ctx):
    """DVE multiply into an (h,s) prod buffer; reduction over s split
    between DVE (h < K_DVE, one strided tensor_reduce) and ACT (h >=
    K_DVE, per-h activation accumulate), so the two engines share the
    reduce load and DVE stays near its multiply-only floor."""
    f32 = mybir.dt.float32
    k_dve = int(os.environ.get("KERNEL_KDVE", "26"))
    bcast_pool = ctx.enter_context(tc.tile_pool(name="bcast", bufs=1))
    enc_pool = ctx.enter_context(tc.tile_pool(name="enc", bufs=2))
    prod_pool = ctx.enter_context(tc.tile_pool(name="prod", bufs=2))
    small_pool = ctx.enter_context(tc.tile_pool(name="small", bufs=6))
    scr_pool = ctx.enter_context(tc.tile_pool(name="scr", bufs=4))

    rel_bcast = bcast_pool.tile([P, F], f32)
    _replicate_row(nc, rel_bcast, rel_h, [cs * H for cs in S_CHUNKS])
    rel_3d = rel_bcast[:, :].rearrange("p (s h) -> p s h", h=H)

    for ib in range(N_BTILES):
        accs = []
        s0 = 0
        for cs in S_CHUNKS:
            et = enc_pool.tile([P, cs, H], f32, tag="enc")
            nc.sync.dma_start(
                out=et[:, :, :],
                in_=enc_h[ib * P : (ib + 1) * P, s0 : s0 + cs, :],
            )
            pt = prod_pool.tile([P, H, cs], f32, tag="prod")
            nc.vector.tensor_mul(
                pt[:, :, :].rearrange("p h s -> p s h"),
                et[:, :, :],
                rel_3d[:, s0 : s0 + cs, :],
            )
            acc = small_pool.tile([P, H], f32, tag="acc")
            nc.vector.reduce_sum(
                out=acc[:, 0:k_dve],
                in_=pt[:, 0:k_dve, :],
                axis=mybir.AxisListType.X,
            )
            for h in range(k_dve, H):
                scratch = scr_pool.tile([P, cs], f32, tag=f"scr{h % 4}")
                nc.scalar.activation(
                    out=scratch[:, :],
                    in_=pt[:, h, :],
                    func=mybir.ActivationFunctionType.Copy,
                    bias=0.0,
                    scale=1.0,
                    accum_out=acc[:, h : h + 1],
                )
            accs.append(acc)
            s0 += cs
        ot = small_pool.tile([P, H], f32, tag="ot")
        nc.vector.tensor_add(ot[:, :], accs[0][:, :], accs[1][:, :])
        nc.sync.dma_start(out=out_h[ib * P : (ib + 1) * P, :], in_=ot[:, :])


def _get_nc():
    key = PATH
    if key in _NC_CACHE:
        return _NC_CACHE[key]
    from contextlib import ExitStack

    nc = bacc.Bacc("TRN2")
    enc_h = nc.dram_tensor("enc", [B, S_PER, H], mybir.dt.float32, kind="ExternalInput")
    rel_h = nc.dram_tensor("rel", [1, F], mybir.dt.float32, kind="ExternalInput")
    out_h = nc.dram_tensor("out", [B, H], mybir.dt.float32, kind="ExternalOutput")
    with ExitStack() as ctx:
        tc = ctx.enter_context(tile.TileContext(nc))
        if PATH == "A":
            _build_path_a(nc, tc, enc_h, rel_h, out_h, ctx)
        elif PATH == "C":
            _build_path_c(nc, tc, enc_h, rel_h, out_h, ctx)
        else:
            _build_path_b(nc, tc, enc_h, rel_h, out_h, ctx)
    nc.finalize()  # Bacc: splits multi-sem waits, allocates registers
    _NC_CACHE[key] = nc
    return nc


def kernel(stock_idx, encoded_states, relationship_matrix, W1, b1, W2, b2):
    global LAST_RESULT
    idx = int(np.asarray(stock_idx))
    enc = np.ascontiguousarray(np.asarray(encoded_states, dtype=np.float32))
    relationships = np.asarray(relationship_matrix[idx], dtype=np.float32)  # [S, H]
    W1 = np.asarray(W1, dtype=np.float32)
    W2 = np.asarray(W2, dtype=np.float32)
    b1 = np.asarray(b1, dtype=np.float32)
    b2 = np.asarray(b2, dtype=np.float32)

    # Tiny 2-layer MLP + mask on host (0.006% of total FLOPs).
    h = np.maximum(relationships @ W1.T + b1, 0.0)
    rel_enc = (h @ W2.T + b2).astype(np.float32)  # [S, H]
    rel_enc[idx, :] = 0.0

    in_maps = []
    for c in range(N_CORES):
        sl = slice(c * S_PER, (c + 1) * S_PER)
        in_maps.append(
            {
                "enc": np.ascontiguousarray(enc[:, sl, :]),
                "rel": np.ascontiguousarray(rel_enc[sl, :]).reshape(1, F),
            }
        )

    if not TRACE:
        # This axon client lacks antenv.axon_hooks; a BASS_TRACE=1 env var
        # would send run_bass_kernel_spmd down that broken import path.
        os.environ["BASS_NEVER_TRACE"] = "1"
    nc = _get_nc()
    res = run_bass_kernel_spmd(
        nc,
        in_maps,
        core_ids=list(range(N_CORES)),
        trace=TRACE,
        trace_cores=list(range(N_CORES)) if TRACE else None,
    )
    LAST_RESULT = res
    out = np.zeros((B, H), dtype=np.float32)
    for r in res.results:
        out += r["out"]
    return out
